# revision 51
# baseline (speedup 1.0000x reference)
"""Trainium2 Bass kernel for nn_MultiHeadAttention_7584912245188.

Reference computes (no softmax!):
    qkv = x @ Wqkv + bqkv ; split q,k,v ; per head: y = (q k^T / sqrt(D)) v
    out = y @ Wff + bff

No softmax => attention is linear and reassociates: (Q K^T) V = Q (K^T V).
With X_aug = [X | 1] ([N, 97]) and G = X_aug^T X_aug ([97, 97]) the module
collapses to out = X_aug @ Wfin computed on device as:
    R = G @ Qcat                  [97, 576]  (3 matmuls; Q_h = s Wv_h Wff_h)
    Wfin = sum_h P_h R_h + bff    [97, 96]   (7-matmul group; P_h = Wq_h Wk_h^T)
    out rows {8p+j} = X @ Wfin    (8 matmuls via host-shipped X^T)

v3 (10574ns) over the 10956ns baseline:
  - the WHOLE batch ships once in fp8 for the Gram (each core redundantly
    computes the full-batch Gram; G is diagonally dominated so fp8 inputs
    keep the end-to-end rel err 8.3e-3 < 2e-2) and the Gram runs as 8
    DoubleRow fp8 matmuls (0.5 cyc/row, two 128-row chunks per matmul;
    feature dim padded to 112: dual-fp8 ldweights needs free dim % 16).
  - X^T for the finals is host-transposed and DMA'd (fp16), removing the
    on-device PE transposes, the identity build, and the XT staging copies.
  - x splits 10/6: ten chunks on the sync-HWDGE slot (first bytes at
    1966ns), six on the Pool SWDGE queue whose +900ns completion sem lands
    exactly as the PE finishes the first ten chunks' matmuls.
  - HWDGE issue slots serialize ACROSS queues (~630ns each), so only two
    DMAs ride HWDGE up front (x-first, wq); wp takes Act's second slot and
    X^T the second SWDGE prep - each +900ns completion sem lands just
    before its consumer: wq -> R, wp -> Wfin, X^T -> finals.
  - PSUM->SBUF copies: one engine per tile (cross-engine writes to one
    tile serialize +~500ns in the Tile sync model); r splits 288/288
    DVE/Act so Wfin h0/h3 gate evenly; bias matmul sits between h2 and
    h3 to absorb the PE sequencer park on Act's r-copy semaphore.
  - output leaves as two single-writer tiles: first finals group -> DVE
    copy -> SWDGE store (DVE->Pool sem prop is ~50ns cheaper than
    Act->Pool, and the 1038ns desc prep + 650ns DGE eat the earlier-ready
    half), second group -> Act copy -> sync-HWDGE store.
Dead ends measured: kv_writeback prepare_only+trigger_dma store (saves
~1.3us on HW but TimelineSim drops per-entry trigger sem credits ->
deadlock), 3-way r split and Act 192+96 r pieces (PE dispatch quanta /
second-copy serialization), column-split Wfin/finals (+800), gcopy/osb
two-writer splits (serialize), merged weight DMAs (completion sem lands
after consumer), store group rebalancing 3/5 and 5/3 (engine-exclusive
transfer ordering), NXA=8/12 (far-gram gate / sub-512B descriptors),
single_packet (no-op in TimelineSim).
"""

import numpy as np
from contextlib import ExitStack

import ml_dtypes
import concourse.bass as bass
import concourse.tile as tile
from concourse import bacc, mybir
from concourse import bass_utils

B, N, E = 4, 2048, 96
H = 6
D = E // H            # 16
EA = E + 1            # 97 (augmented ones column)
NH = N // 2           # 1024 rows per half (per core)
NCH = 8               # finals row chunks (chunk j = rows {8p + j})
NCG = 16              # gram row chunks over the whole batch
SCALE = float(D) ** -0.5
F32 = mybir.dt.float32
F16 = mybir.dt.float16
F8 = mybir.dt.float8e4
NP_F8 = ml_dtypes.float8_e4m3
DR = mybir.MatmulPerfMode.DoubleRow
GP = 112           # gram feature dim padded to %16 for dual-fp8 ldweights

N_WARM = 12           # PE p-state warmup matmuls
WARM_COLS = 128

N_CORES = 8

_NC_CACHE = {}
LAST_RESULTS = None


def _build_nc():
    nc = bacc.Bacc(
        "TRN2", target_bir_lowering=False, debug=False, num_devices=N_CORES,
    )
    xalli = nc.dram_tensor("xall", [N, GP], F8, kind="ExternalInput").ap()
    xti = nc.dram_tensor("xt", [EA, NH], F16, kind="ExternalInput").ap()
    # wpkq cols: Qcat (H*E) | onehot e_last^T (EA) | bff (E), all on row 0+
    wqi = nc.dram_tensor("wpkq", [EA, H * E + EA + E], F16, kind="ExternalInput").ap()
    wpi = nc.dram_tensor("wpkp", [EA, H * EA], F16, kind="ExternalInput").ap()
    outd = nc.dram_tensor("out", [128, NCH * E], F16, kind="ExternalOutput").ap()

    with tile.TileContext(nc) as tc, ExitStack() as ctx:
        consts = ctx.enter_context(tc.tile_pool(name="consts", bufs=1))
        big = ctx.enter_context(tc.tile_pool(name="big", bufs=1))
        small = ctx.enter_context(tc.tile_pool(name="small", bufs=1))
        outp = ctx.enter_context(tc.tile_pool(name="outp", bufs=1))
        ps_gw = ctx.enter_context(tc.tile_pool(name="ps_gw", bufs=1, space="PSUM"))
        ps_r = ctx.enter_context(tc.tile_pool(name="ps_r", bufs=2, space="PSUM"))
        ps_o = ctx.enter_context(tc.tile_pool(name="ps_o", bufs=2, space="PSUM"))

        # --- DMA schedule. Issue order fixes each transfer's slot on the
        # shared DMA engines; the +900ns completion sem then lands just
        # before each consumer needs the data.
        # x: first ten chunks on the fastest (sync HWDGE) slot, the rest on
        # the Pool SWDGE queue (prep pipelines on the idle Pool engine); wq
        # and wp on Act's two HWDGE slots, XT on the second SWDGE prep.
        NXA = 10  # sync-slot chunks; SWDGE half carries the rest
        xr = xalli.rearrange("(p j) e -> p j e", j=NCG)
        XA = big.tile([128, NXA, GP], F8)
        nc.sync.dma_start(out=XA[:], in_=xr[:, 0:NXA, :])
        XB = big.tile([128, NCG - NXA, GP], F8)
        nc.gpsimd.dma_start(out=XB[:], in_=xr[:, NXA:NCG, :])
        wq = consts.tile([EA, H * E + EA + E], F16)
        nc.scalar.dma_start(out=wq[:], in_=wqi)
        wp = consts.tile([EA, H * EA], F16)
        nc.scalar.dma_start(out=wp[:], in_=wpi)
        XT = big.tile([EA, NCH, 128], F16)
        nc.gpsimd.dma_start(out=XT[:], in_=xti.rearrange("e (j p) -> e j p", p=128))

        # --- G = X_aug^T X_aug over the whole batch: 8 DoubleRow fp8
        # matmuls, each reducing two 128-row chunks (feature dim padded to
        # GP=112 with zero columns: dual-fp8 ldweights needs free dim % 16)
        g_ps = ps_gw.tile([GP, GP], F32, tag="gw", name="g")
        for c in range(NCG // 2):
            src_t = XA if 2 * c < NXA else XB
            cc = c - (0 if 2 * c < NXA else NXA // 2)
            xc = src_t[:, 2 * cc : 2 * cc + 2, :]
            nc.tensor.matmul(
                g_ps[:], lhsT=xc, rhs=xc,
                start=(c == 0), stop=(c == NCG // 2 - 1), perf_mode=DR,
            )
        g_h = small.tile([EA, EA], F16)
        nc.vector.tensor_copy(out=g_h[:], in_=g_ps[0:EA, 0:EA])

        # --- R = G @ Qcat in two 288-col pieces; each piece's PSUM->SBUF
        # copy starts as soon as its matmul retires (DVE then Act; GPSIMD
        # cannot read PSUM). DVE's piece gates Wfin h0, Act's lands just as
        # h3 needs it.
        r_h = small.tile([EA, H * E], F16)
        rsplit = [(0, 3 * E), (3 * E, H * E)]
        rcp = [nc.vector.tensor_copy, nc.scalar.copy]
        for i, (lo, hi) in enumerate(rsplit):
            r_ps = ps_r.tile([EA, hi - lo], F32, tag="r", name=f"r{i}")
            nc.tensor.matmul(
                r_ps[:], lhsT=g_h[:], rhs=wq[0:EA, lo:hi],
                start=True, stop=True,
            )
            rcp[i](out=r_h[:, lo:hi], in_=r_ps[:])

        # --- Wfin = sum_h P_h R_h + e_last bff^T (one accum group, PSUM
        # bank shared with G - dead after g_h)
        # bias mm between h2 and h3: its 40ns fill the window where the PE
        # sequencer would otherwise park waiting for Act's r-copy semaphore
        wf_ps = ps_gw.tile([EA, E], F32, tag="gw", name="wf")
        for h in range(H):
            if h == 3:
                nc.tensor.matmul(
                    wf_ps[:],
                    lhsT=wq[0:1, H * E : H * E + EA],
                    rhs=wq[0:1, H * E + EA : H * E + EA + E],
                    start=False, stop=False,
                )
            nc.tensor.matmul(
                wf_ps[:],
                lhsT=wp[0:EA, EA * h : EA * (h + 1)],
                rhs=r_h[:, E * h : E * (h + 1)],
                start=(h == 0), stop=(h == H - 1),
            )
        wf_h = small.tile([EA, E], F16)
        nc.vector.tensor_copy(out=wf_h[:], in_=wf_ps[:])

        # --- finals: out rows {8p+j} = X_chunk @ Wfin (lhsT = host-shipped
        # X^T). Each 4-chunk group's copies go 3-way; the first group leaves
        # on the sync HWDGE slot, the second on the Pool SWDGE path.
        # first finals group -> DVE copy -> SWDGE store (long prep eats the
        # earlier-ready data); second group -> Act copy -> sync HWDGE store
        osbA = outp.tile([128, 4 * E], F16)
        osbB = outp.tile([128, 4 * E], F16)
        ocp = [nc.vector.tensor_copy, nc.scalar.copy]
        for g, osb_g in enumerate((osbA, osbB)):
            og = ps_o.tile([128, 4, E], F32, tag="og", name=f"og{g}")
            for j4 in range(4):
                nc.tensor.matmul(
                    og[:, j4, :], lhsT=XT[:, 4 * g + j4, :], rhs=wf_h[:],
                    start=True, stop=True,
                )
            ocp[g](out=osb_g[:], in_=og[:].rearrange("p a b -> p (a b)"))
        nc.gpsimd.dma_start(out=outd[:, 0 : 4 * E], in_=osbA[:])
        nc.sync.dma_start(out=outd[:, 4 * E : 8 * E], in_=osbB[:])

    nc.compile()
    return nc


def get_nc():
    if "nc" not in _NC_CACHE:
        _NC_CACHE["nc"] = _build_nc()
    return _NC_CACHE["nc"]


def _host_weights(Wqkv, bqkv, Wff, bff):
    waug = np.concatenate(
        [np.asarray(Wqkv, np.float64), np.asarray(bqkv, np.float64)[None, :]], axis=0
    )
    Wq, Wk, Wv = waug[:, 0:E], waug[:, E : 2 * E], waug[:, 2 * E : 3 * E]
    Wff = np.asarray(Wff, np.float64)
    wpkq = np.zeros((EA, H * E + EA + E), np.float16)
    wpkp = np.zeros((EA, H * EA), np.float16)
    for h in range(H):
        hd = slice(h * D, (h + 1) * D)
        wpkp[:, EA * h : EA * (h + 1)] = (
            Wq[:, hd] @ Wk[:, hd].T
        ).T.astype(np.float16)
        wpkq[0:EA, E * h : E * (h + 1)] = (
            SCALE * (Wv[:, hd] @ Wff[hd, :])
        ).astype(np.float16)
    wpkq[0, H * E + E] = 1.0  # row-0 onehot block = e_last^T
    wpkq[0, H * E + EA : H * E + EA + E] = np.asarray(bff, np.float16)
    return {"wpkq": wpkq, "wpkp": wpkp}


def make_in_maps(x, Wqkv, bqkv, Wff, bff):
    x = np.asarray(x, np.float32)
    w = _host_weights(Wqkv, bqkv, Wff, bff)
    in_maps = []
    xall_b = []
    for b in range(B):
        xaug = np.zeros((N, GP), np.float32)
        xaug[:, 0:E] = x[b]
        xaug[:, E] = 1.0
        xall_b.append(xaug.astype(NP_F8))
    for c in range(N_CORES):
        b, h = divmod(c, 2)
        mine = np.ones((NH, EA), np.float16)
        mine[:, 0:E] = x[b, h * NH : (h + 1) * NH].astype(np.float16)
        # xt[e, j*128+p] = mine[8p+j, e]
        xt = np.ascontiguousarray(
            mine.reshape(128, NCH, EA).transpose(2, 1, 0).reshape(EA, NH)
        )
        m = {"xall": xall_b[b], "xt": xt}
        m.update(w)
        in_maps.append(m)
    return in_maps


def assemble(results):
    out = np.empty((B, N, E), np.float32)
    for c in range(N_CORES):
        b, h = divmod(c, 2)
        half = results[c]["out"].reshape(128, NCH, E).astype(np.float32)
        out[b, h * NH : (h + 1) * NH] = half.reshape(NH, E)
    return out


def kernel(x, Wqkv, bqkv, Wff, bff):
    global LAST_RESULTS
    nc = get_nc()
    in_maps = make_in_maps(x, Wqkv, bqkv, Wff, bff)
    res = bass_utils.run_bass_kernel_spmd(
        nc, in_maps, core_ids=list(range(N_CORES))
    )
    LAST_RESULTS = res
    return assemble(res.results)
